# revision 6
# baseline (speedup 1.0000x reference)
"""BERT self-attention (B=8, S=2048, H=768, NH=12) on 8 NeuronCores.

Sharding: pure data-parallel over the batch dim — core c computes batch
element c end-to-end (weights replicated). No collectives needed.

Per-core algorithm (all matmuls in bf16, fp32 accumulation):
  1. Load X [S, H] fp32, cast to bf16, xbar-transpose to X^T (h on
     partitions) so the PE can contract over h.
  2. Same for Wq/Wk/Wv -> W^T (h on partitions).
  3. Q^T = Wq X^T + bq  (layout [jout, s], i.e. d on partitions per head)
     K^T likewise; V = X Wv^T + bv kept natural [s, jout], stored per
     head as V~ = [V_h | 1] (extra ones column).
  4. Per head, per 1024-wide i-half, per 128-row j-tile:
       scores^T[j, i] = K_h^T.T @ Q_h^T          (PSUM, fp32)
       e = exp(scores^T/8 + mask_j)              (ACT, PSUM->SBUF bf16)
       ctx[i, 0:64] += e.T @ V_h ; ctx[i, 64] += e.T @ 1   (one matmul
         per 128-i slice with stationary=e, moving=[V|1]; the ones
         column accumulates the softmax denominator for free)
     then ctx_norm = ctx[:, 0:64] * (1/ctx[:, 64]) -> DMA to DRAM.
  Softmax max-subtraction is skipped: scores are bounded (|s| < ~6 for
  this distribution) so exp is safe in fp32.
"""

import numpy as np

try:
    import concourse.bass as bass
except ImportError:  # pragma: no cover - path fallback for fresh dirs
    import sys

    sys.path.insert(0, "/opt/trn_rl_repo")
    import concourse.bass as bass

import concourse.bacc as bacc
import concourse.mybir as mybir
import concourse.tile as tile
from concourse.bass_utils import run_bass_kernel_spmd

B, S, H, NH = 8, 2048, 768, 12
HD = H // NH  # 64
HC = H // 128  # 6 h-chunks
ST = S // 128  # 16 s-tiles
N_CORES = 8
F32 = mybir.dt.float32
BF16 = mybir.dt.bfloat16
FA = mybir.ActivationFunctionType
ADD = mybir.AluOpType.add
MULT = mybir.AluOpType.mult


def _emit(nc, tc):
    x = nc.dram_tensor("x", [S, H], F32, kind="ExternalInput").ap()
    mask = nc.dram_tensor("mask", [S], F32, kind="ExternalInput").ap()
    wq = nc.dram_tensor("wq", [H, H], F32, kind="ExternalInput").ap()
    wk = nc.dram_tensor("wk", [H, H], F32, kind="ExternalInput").ap()
    wv = nc.dram_tensor("wv", [H, H], F32, kind="ExternalInput").ap()
    bq = nc.dram_tensor("bq", [H], F32, kind="ExternalInput").ap()
    bk = nc.dram_tensor("bk", [H], F32, kind="ExternalInput").ap()
    bv = nc.dram_tensor("bv", [H], F32, kind="ExternalInput").ap()
    out = nc.dram_tensor("out", [S, H], F32, kind="ExternalOutput").ap()

    from contextlib import ExitStack

    whole = ExitStack()
    const = whole.enter_context(tc.tile_pool(name="const", bufs=1))
    big = whole.enter_context(tc.tile_pool(name="big", bufs=1))
    phase1 = ExitStack()
    xstage = phase1.enter_context(tc.tile_pool(name="xstage", bufs=3))
    wstage = phase1.enter_context(tc.tile_pool(name="wstage", bufs=2))
    projp = phase1.enter_context(tc.tile_pool(name="projp", bufs=2, space="PSUM"))

    # --- constants ---
    mask_sb = const.tile([128, ST], F32)
    zconst = const.tile([1, 512], BF16)
    nc.vector.memset(zconst, 0.0)
    bq_sb = const.tile([128, HC], F32)
    bk_sb = const.tile([128, HC], F32)
    bv_row = const.tile([1, H], F32)
    bv_bc = const.tile([128, H], F32)
    with nc.allow_non_contiguous_dma(reason="tiny one-time per-partition loads"):
        nc.sync.dma_start(out=mask_sb, in_=mask.rearrange("(f p) -> p f", p=128))
        nc.sync.dma_start(out=bq_sb, in_=bq.rearrange("(f p) -> p f", p=128))
        nc.sync.dma_start(out=bk_sb, in_=bk.rearrange("(f p) -> p f", p=128))
    nc.sync.dma_start(out=bv_row, in_=bv.rearrange("(a h) -> a h", a=1))
    nc.gpsimd.partition_broadcast(bv_bc, bv_row, 128)

    # --- big persistent tensors ---
    XT = big.tile([128, ST * HC * 128], BF16)  # X^T as (t, c, s)
    WTq = big.tile([128, HC * HC * 128], BF16)  # W^T as (t, c, j)
    WTk = big.tile([128, HC * HC * 128], BF16)
    WTv = big.tile([128, HC * HC * 128], BF16)
    QT = big.tile([128, HC * S], BF16)  # (c, s)
    KT = big.tile([128, HC * S], BF16)
    VT = big.tile([128, NH * ST * 65], BF16)  # (h, t, [v|1])

    XT4 = XT.rearrange("p (t c s) -> p t c s", t=ST, c=HC)
    WTq4 = WTq.rearrange("p (t c j) -> p t c j", t=HC, c=HC)
    WTk4 = WTk.rearrange("p (t c j) -> p t c j", t=HC, c=HC)
    WTv4 = WTv.rearrange("p (t c j) -> p t c j", t=HC, c=HC)
    QT3 = QT.rearrange("p (c s) -> p c s", c=HC)
    KT3 = KT.rearrange("p (c s) -> p c s", c=HC)
    VT4 = VT.rearrange("p (h t o) -> p h t o", h=NH, t=ST)

    # ones columns of V~ (softmax denominator trick)
    nc.vector.memset(VT4[:, :, :, 64], 1.0)

    # --- load + cast + transpose X ---
    for t in range(ST):
        xnat = xstage.tile([128, H], F32, tag="xnat")
        nc.sync.dma_start(out=xnat, in_=x[t * 128 : (t + 1) * 128, :])
        xbf = xstage.tile([128, H], BF16, tag="xbf")
        nc.vector.tensor_copy(out=xbf, in_=xnat)
        nc.sync.dma_start_transpose(XT4[:, t], xbf)

    # --- load + cast + transpose W ---
    for w_dram, WT4 in ((wq, WTq4), (wk, WTk4), (wv, WTv4)):
        for t in range(HC):
            wnat = wstage.tile([128, H], F32, tag="wnat")
            nc.sync.dma_start(out=wnat, in_=w_dram[t * 128 : (t + 1) * 128, :])
            wbf = wstage.tile([128, H], BF16, tag="wbf")
            nc.vector.tensor_copy(out=wbf, in_=wnat)
            nc.sync.dma_start_transpose(WT4[:, t], wbf)

    # --- Q^T / K^T projections: out [jout-chunk partitions, s free] ---
    for WT4, bsb, DST3 in ((WTq4, bq_sb, QT3), (WTk4, bk_sb, KT3)):
        for cc in range(HC):  # jout chunk
            for s4 in range(4):  # 512-wide s chunk
                ps = projp.tile([128, 512], F32, tag="proj")
                for hc in range(HC):
                    nc.tensor.matmul(
                        ps,
                        lhsT=WT4[:, cc, hc, :],
                        rhs=XT4[:, 4 * s4 : 4 * s4 + 4, hc, :],
                        start=(hc == 0),
                        stop=(hc == HC - 1),
                    )
                nc.vector.tensor_scalar(
                    DST3[:, cc, s4 * 512 : (s4 + 1) * 512],
                    ps,
                    bsb[:, cc : cc + 1],
                    None,
                    ADD,
                )

    # --- V projection (natural layout) + bias, packed per head with ones ---
    for t in range(ST):
        for n0, nw in ((0, 512), (512, 256)):
            ps = projp.tile([128, 512], F32, tag="proj")
            for hc in range(HC):
                nc.tensor.matmul(
                    ps[:, :nw],
                    lhsT=XT4[:, t, hc, :],
                    rhs=WTv4[:, n0 // 128 : (n0 + nw) // 128, hc, :],
                    start=(hc == 0),
                    stop=(hc == HC - 1),
                )
            for h in range(n0 // HD, (n0 + nw) // HD):
                off = h * HD - n0
                nc.vector.tensor_tensor(
                    out=VT4[:, h, t, 0:HD],
                    in0=ps[:, off : off + HD],
                    in1=bv_bc[:, h * HD : (h + 1) * HD],
                    op=ADD,
                )

    phase1.close()  # releases proj psum + staging pools (space reuse)

    # --- attention ---
    phase2 = ExitStack()
    scp = phase2.enter_context(tc.tile_pool(name="scp", bufs=2, space="PSUM"))
    ctxp = phase2.enter_context(tc.tile_pool(name="ctxp", bufs=4, space="PSUM"))
    esp = phase2.enter_context(tc.tile_pool(name="esp", bufs=3))
    osp = phase2.enter_context(tc.tile_pool(name="osp", bufs=8))

    for h in range(NH):
        cc = h // 2
        po = (h % 2) * 64
        for half in range(2):
            ctxA = ctxp.tile([128, 512], F32, tag="ctx")
            ctxB = ctxp.tile([128, 512], F32, tag="ctx")
            # Zero-fill both banks with a K=1 dummy matmul (start=True clears
            # has_written for the whole bank).  The PV matmuls below then all
            # run with start=False: first write per element lands on
            # pending-zero (overwrite), later ones accumulate.  This allows 8
            # independent 65-column accumulation regions packed into 2 banks.
            for ctx_t in (ctxA, ctxB):
                nc.tensor.matmul(
                    ctx_t,
                    lhsT=zconst[:, 0:128],
                    rhs=zconst[:, 0:512],
                    start=True,
                    stop=True,
                )
            for j in range(ST):
                sc = scp.tile([128, 1024], F32, tag="sc")
                lhsT = KT3[po : po + 64, cc, j * 128 : (j + 1) * 128]
                for n in range(2):
                    i0 = half * 1024 + n * 512
                    nc.tensor.matmul(
                        sc[:, n * 512 : (n + 1) * 512],
                        lhsT=lhsT,
                        rhs=QT3[po : po + 64, cc, i0 : i0 + 512],
                        start=True,
                        stop=True,
                    )
                es = esp.tile([128, 1024], BF16, tag="es")
                nc.scalar.activation(
                    es, sc, FA.Exp, bias=mask_sb[:, j : j + 1], scale=0.125
                )
                for i8 in range(8):
                    dst = (
                        ctxA[:, i8 * 65 : (i8 + 1) * 65]
                        if i8 < 7
                        else ctxB[:, 0:65]
                    )
                    nc.tensor.matmul(
                        dst,
                        lhsT=es[:, i8 * 128 : (i8 + 1) * 128],
                        rhs=VT4[:, h, j, :],
                        start=False,
                        stop=(j == ST - 1),
                        skip_group_check=True,
                    )
            for i8 in range(8):
                cap = ctxA[:, i8 * 65 : (i8 + 1) * 65] if i8 < 7 else ctxB[:, 0:65]
                rec = osp.tile([128, 1], F32, tag="rec")
                nc.vector.reciprocal(rec, cap[:, 64:65])
                ot = osp.tile([128, HD], F32, tag="ot")
                nc.vector.tensor_scalar(ot, cap[:, 0:HD], rec, None, MULT)
                it = half * 8 + i8
                nc.sync.dma_start(
                    out=out[it * 128 : (it + 1) * 128, h * HD : (h + 1) * HD],
                    in_=ot,
                )
    phase2.close()
    whole.close()


_CACHED_NC = None


def _get_program():
    global _CACHED_NC
    if _CACHED_NC is None:
        nc = bacc.Bacc(
            "TRN2",
            target_bir_lowering=False,
            debug=False,
            enable_asserts=False,
            num_devices=N_CORES,
        )
        with tile.TileContext(nc) as tc:
            _emit(nc, tc)
        nc.compile()
        _CACHED_NC = nc
    return _CACHED_NC


def make_in_maps(hidden_states, attention_mask, Wq, bq, Wk, bk, Wv, bv):
    def f32(a):
        return np.ascontiguousarray(np.asarray(a, dtype=np.float32))

    hidden_states = f32(hidden_states)
    attention_mask = f32(attention_mask).reshape(B, S)
    shared = {
        "wq": f32(Wq),
        "wk": f32(Wk),
        "wv": f32(Wv),
        "bq": f32(bq),
        "bk": f32(bk),
        "bv": f32(bv),
    }
    return [
        {"x": hidden_states[c], "mask": attention_mask[c], **shared}
        for c in range(N_CORES)
    ]


def kernel(hidden_states, attention_mask, Wq, bq, Wk, bk, Wv, bv, **run_kwargs):
    nc = _get_program()
    in_maps = make_in_maps(hidden_states, attention_mask, Wq, bq, Wk, bk, Wv, bv)
    res = run_bass_kernel_spmd(nc, in_maps, list(range(N_CORES)), **run_kwargs)
    out = np.stack([res.results[c]["out"] for c in range(N_CORES)])
    kernel.last_results = res
    return out


if __name__ == "__main__":
    import jax

    key = jax.random.key(0)
    ks = jax.random.split(key, 7)
    hs = np.asarray(jax.random.normal(ks[0], (B, S, H)), dtype=np.float32)
    am = np.zeros((B, 1, 1, S), np.float32)
    mk = lambda k: np.asarray(jax.random.normal(k, (H, H)), np.float32) * 0.02
    o = kernel(hs, am, mk(ks[1]), np.zeros(H, np.float32), mk(ks[2]),
               np.zeros(H, np.float32), mk(ks[3]), np.zeros(H, np.float32))
    print(o.shape, o.dtype)
